# revision 14
# baseline (speedup 1.0000x reference)
"""Trainium2 Bass kernel for nn_ConstGCN.

Math note: in the reference, the attention score s[b,i] is constant along
the softmax axis j, and softmax is shift-invariant, so
p = softmax(s + mask) = softmax(mask) and p.sum(axis=2) == 1 (to ~1e-6 in
f32).  The output therefore collapses to

    out = relu(text + mean_k(emb_table[const_labels[...,k]]) @ fc_W.T + fc_b)

which depends on neither const_mat nor attn_W/attn_b.  The embedding + fc
fuse into a single table M2 = (emb_table @ fc_W.T)/8, so

    out[b,l,:] = relu(text[b,l,:] + sum_k M2[labels[b,l,k], :] + fc_b)

Input marshalling on host: the integer labels [pos, 8] are re-encoded as
per-position class-count vectors (np.bincount; counts in 0..8 are exact in
fp8e4m3), shipped transposed as [class, pos] so the device consumes them
directly as the matmul stationary.  Class row 100 is constant 1 and M2 row
100 = fc_b, folding the bias into the matmul; rows 101..127 are zero.
text is shipped as bf16, the output as fp16 (tolerance is 2e-2; these add
~4e-3).

On device (per core, data-parallel over batch: 2 of 16 batches = 4096
positions, in 4 super-chunks of 1024 positions = 512 KiB DMA transfers,
each computed as 2 sub-chunks of 512 positions):
  - PE: identity matmuls stream the text sub-chunk into PSUM (start=True),
    then four fp8xbf16 matmuls accumulate counts.T @ M2 on top
  - relu + cast f32->fp16 evicts PSUM, split between ACT and DVE
  - DMA: text in alternates sync-HWDGE / gpsimd-SWDGE, out alternates
    scalar-HWDGE / gpsimd-SWDGE so both directions stream concurrently;
    constants load on the scalar ring so text starts at t=0
const_mat (256 MiB) is never read.
"""

import numpy as np
import ml_dtypes

B, L, D = 16, 2048, 256
CN, K = 100, 8
NCLS = 128         # 100 label classes + bias class 100 (M2 row 100 = fc_b)
NCORES = 8
POS = (B // NCORES) * L          # 4096 positions per core
CHUNK = 512                      # positions per compute sub-chunk
NCHUNK = POS // CHUNK            # 8
Q = CHUNK // 128                 # 4 position-groups of 128 per sub-chunk
NSUP = 4                         # DMA super-chunks (1024 positions each)
SUB = NCHUNK // NSUP             # sub-chunks per super-chunk (2)

_compiled = None


def _build():
    import concourse.bacc as bacc
    import concourse.mybir as mybir
    from concourse.tile import TileContext

    f32 = mybir.dt.float32
    fp16 = mybir.dt.float16
    bf16 = mybir.dt.bfloat16
    fp8 = mybir.dt.float8e4

    nc = bacc.Bacc("TRN2", target_bir_lowering=False)

    text_d = nc.dram_tensor("text", [128, NCHUNK * Q * D], bf16,
                            kind="ExternalInput")
    ct_d = nc.dram_tensor("ct", [128, NCHUNK * Q * 128], fp8,
                          kind="ExternalInput")
    m2_d = nc.dram_tensor("m2", [NCLS, D], bf16, kind="ExternalInput")
    out_d = nc.dram_tensor("out", [128, NCHUNK * Q * D], fp16,
                           kind="ExternalOutput")

    ident_d = nc.inline_tensor(np.eye(128, dtype=ml_dtypes.bfloat16),
                               name="ident")

    text_v = text_d
    out_v = out_d
    SUPW = SUB * Q * D               # free words per super-chunk (2048)

    with TileContext(nc) as tc:
        with (
            tc.tile_pool(name="const", bufs=1) as cpool,
            tc.tile_pool(name="in", bufs=2) as ipool,
            tc.tile_pool(name="res", bufs=2) as rpool,
            tc.tile_pool(name="ps", bufs=4, space="PSUM") as pst,
        ):
            # constants on the scalar ring; big streams start immediately
            ident_sb = cpool.tile([128, 128], bf16)
            nc.scalar.dma_start(out=ident_sb[:, :], in_=ident_d[:, :])
            m2_sb = cpool.tile([NCLS, D], bf16)
            nc.scalar.dma_start(out=m2_sb[:, :], in_=m2_d[:, :])
            # all count-vectors up front in one SWDGE transfer (512 KiB)
            ct_sb = cpool.tile([128, NCHUNK * Q * 128], fp8)
            ct_v = ct_sb.rearrange("p (n x) -> p n x", n=NCHUNK)
            nc.gpsimd.dma_start(out=ct_sb[:, :], in_=ct_d[:, :])

            for s in range(NSUP):
                text_t = ipool.tile([128, SUPW], bf16, tag="text")
                tdma = nc.sync if s % 2 == 0 else nc.gpsimd
                tdma.dma_start(out=text_t[:, :],
                               in_=text_v[:, s * SUPW:(s + 1) * SUPW])

                res = rpool.tile([128, SUB * Q * D], fp16, tag="res")
                for u in range(SUB):
                    n = s * SUB + u
                    acc = pst.tile([128, Q * D], f32, tag="acc")
                    # matmul free size caps at 512 (one PSUM bank)
                    for h in range(2):
                        w = u * Q * D + h * 512
                        nc.tensor.matmul(acc[:, h * 512:(h + 1) * 512],
                                         lhsT=ident_sb[:, :],
                                         rhs=text_t[:, w:w + 512],
                                         start=True, stop=False)
                    for q in range(Q):
                        nc.tensor.matmul(
                            acc[:, q * D:(q + 1) * D],
                            lhsT=ct_v[:, n, q * 128:(q + 1) * 128],
                            rhs=m2_sb[:, :],
                            start=False, stop=True,
                        )
                    half = Q * D // 2
                    r0 = u * Q * D
                    nc.scalar.activation(res[:, r0:r0 + half],
                                         acc[:, :half],
                                         mybir.ActivationFunctionType.Relu)
                    nc.vector.tensor_scalar_max(out=res[:, r0 + half:r0 + Q * D],
                                                in0=acc[:, half:],
                                                scalar1=0.0)
                odma = nc.scalar if s % 2 == 0 else nc.gpsimd
                odma.dma_start(out=out_v[:, s * SUPW:(s + 1) * SUPW],
                               in_=res[:, :])

    nc.finalize()
    return nc


def _get_compiled():
    global _compiled
    if _compiled is None:
        _compiled = _build()
    return _compiled


def _host_prep(text, const_labels, emb_table, fc_W, fc_b):
    """Marshal full inputs -> per-core in_maps."""
    # fused gather table: row c (c<CN) = (emb_table @ fc_W.T)[c]/8,
    # row 100 = fc_b (count row 100 is constant 1), rows 101..127 zero
    m2 = np.zeros((NCLS, D), dtype=np.float64)
    m2[:CN] = emb_table.astype(np.float64) @ fc_W.T.astype(np.float64) * 0.125
    m2[CN] = fc_b
    m2 = m2.astype(ml_dtypes.bfloat16)

    # label -> count-vector encoding (counts 0..8, exact in fp8e4m3)
    lab = np.ascontiguousarray(const_labels.reshape(B * L, K)).astype(np.int64)
    ids = (np.arange(B * L, dtype=np.int64) * CN)[:, None] + lab
    counts = np.bincount(ids.ravel(), minlength=B * L * CN).reshape(B * L, CN)
    # layout per core: [class, n, q*128 + p] with pos = n*512 + p*4 + q
    cc = counts.reshape(NCORES, NCHUNK, 128, Q, CN)
    ct = np.zeros((NCORES, NCLS, NCHUNK, Q, 128), dtype=np.float32)
    ct[:, :CN] = cc.transpose(0, 4, 1, 3, 2)
    ct[:, CN] = 1.0
    ct = ct.reshape(NCORES, NCLS, NCHUNK * Q * 128).astype(
        ml_dtypes.float8_e4m3fn)

    text16 = np.ascontiguousarray(text.reshape(B * L, D)).astype(
        ml_dtypes.bfloat16)
    # partition-major per core: [128, n*q*d], pos = n*512 + p*4 + q
    text16 = text16.reshape(NCORES, NCHUNK, 128, Q * D).transpose(0, 2, 1, 3)
    text16 = text16.reshape(NCORES, 128, NCHUNK * Q * D)

    in_maps = []
    for c in range(NCORES):
        in_maps.append({
            "text": np.ascontiguousarray(text16[c]),
            "ct": np.ascontiguousarray(ct[c]),
            "m2": m2,
        })
    return in_maps


def kernel(text, const_mat, const_labels, emb_table, attn_W, attn_b,
           fc_W, fc_b):
    from concourse.bass_utils import run_bass_kernel_spmd

    text = np.asarray(text, dtype=np.float32)
    const_labels = np.asarray(const_labels)
    emb_table = np.asarray(emb_table, dtype=np.float32)
    fc_W = np.asarray(fc_W, dtype=np.float32)
    fc_b = np.asarray(fc_b, dtype=np.float32)

    in_maps = _host_prep(text, const_labels, emb_table, fc_W, fc_b)
    nc = _get_compiled()
    r = run_bass_kernel_spmd(nc, in_maps, core_ids=list(range(NCORES)))
    out = np.stack([r.results[c]["out"] for c in range(NCORES)], axis=0)
    # [core, p, (n q d)] -> [core, n, p, q, d]; position = n*512 + p*4 + q
    out = out.reshape(NCORES, 128, NCHUNK, Q * D).transpose(0, 2, 1, 3)
    return out.astype(np.float32).reshape(B, L, D)
